# revision 16
# baseline (speedup 1.0000x reference)
"""Trainium2 Bass kernel for nn_EnsembleClassifier (ragged_sequence).

Strategy
--------
The memory-bound work is masked mean/std pooling over x [2048, 2048, 32]
(~0.5 GB f32). The host masks past-length timesteps to zero and quantizes x
to fp8-e4m3 (verified end-to-end rel err ~5e-3, 4x under the 2e-2 gate),
quartering HBM traffic. Rows are sorted by chunk count ceil(L/128) and dealt
round-robin over the 8 cores; each core gets 16 slots of 16 rows x 32 dims =
512 PSUM columns, with k_slot (up to 16) 128-timestep chunks accumulated in
PSUM per slot.

On each core, per slot:
  - one HWDGE DMA streams the fp8 block [128, kp, 2, 512] from HBM,
  - squares are computed in fp8 split across ScalarE / VectorE / GpSimd
    (by column range, fractions tuned to equalize engine busy time),
  - TensorE reduces timesteps with ones-vector DoubleRow fp8 matmuls
    (2 chunks per instruction, 0.5 cycles/row) accumulating in PSUM,
  - VectorE/ScalarE copy the [1, 512] PSUM results to SBUF; per-slot DMAs
    write them out.

The host combines per-slot sums/sumsqs into masked mean/std, gathers the
last valid timestep from full-precision x, and runs the tiny 3-member MLP
ensemble with exact full-batch BatchNorm in numpy.
"""

import os

import ml_dtypes
import numpy as np

import concourse.bacc as bacc
import concourse.tile as tile
from concourse import mybir
from concourse.bass_utils import run_bass_kernel_spmd

B, T, D = 2048, 2048, 32
P = 128             # SBUF partitions = timesteps per chunk
NCORES = 8
GROUP = 16          # rows per slot (GROUP * D = 512 = PSUM bank f32 width)
NCOLS = GROUP * D   # 512
ROWS_PER_CORE = B // NCORES          # 256
NSLOTS = ROWS_PER_CORE // GROUP      # 16
EPS = 1e-5

USE_DOUBLEROW = True
# square-pass share per engine (by contiguous chunk-pair ranges):
# ScalarE / VectorE / GpSimd, deficit-rounded across slots
SA, SV = 0.40, 0.38

LAST_RESULTS = None


def _build_bass(ks):
    """ks: per-slot chunk counts (even, non-increasing), same for all cores."""
    nslots = len(ks)
    nc = bacc.Bacc()
    f32 = mybir.dt.float32
    f8 = mybir.dt.float8e4
    totalF = sum(k * NCOLS for k in ks)
    xin = nc.dram_tensor("xin", [P, totalF], f8, kind="ExternalInput")
    out = nc.dram_tensor("res", [nslots, 2, NCOLS], f32, kind="ExternalOutput")

    with tile.TileContext(nc) as tc:
        with (
            tc.tile_pool(name="ones", bufs=1) as ones_pool,
            tc.tile_pool(name="data", bufs=4) as data_pool,
            tc.tile_pool(name="sq", bufs=4) as sq_pool,
            tc.tile_pool(name="ps", bufs=4, space="PSUM") as ps_pool,
            tc.tile_pool(name="resp", bufs=4) as res_pool,
        ):
            # pair dim stride must be a multiple of 16 bytes for DoubleRow
            # weight loads (s3_lw_dual_fp8_restrictions), hence [P, 2, 16]
            ones = ones_pool.tile([P, 2, 16], f8)
            nc.vector.memset(ones, 1.0)
            ones_dr = ones[:, :, 0:1]
            ones_plain = ones[:, 0, 0:1]

            # software pipeline state: work deferred from earlier slots
            pending_sq = []    # (slot, kp, sq_tile, psq_tile)
            pending_cp = []    # (slot, psx, psq)
            credit = [0.0, 0.0]

            def emit_mm(ps, src, kp):
                if USE_DOUBLEROW:
                    for j in range(kp):
                        nc.tensor.matmul(
                            ps, ones_dr, src[:, j],
                            start=(j == 0), stop=(j == kp - 1),
                            perf_mode=mybir.MatmulPerfMode.DoubleRow,
                        )
                else:
                    for j in range(kp):
                        for s in range(2):
                            nc.tensor.matmul(
                                ps, ones_plain, src[:, j, s],
                                start=(j == 0 and s == 0),
                                stop=(j == kp - 1 and s == 1),
                            )

            def emit_sq_matmuls():
                i, kp, sqt, psq = pending_sq.pop(0)
                emit_mm(psq, sqt, kp)

            def emit_copies():
                i, psx, psq = pending_cp.pop(0)
                r = res_pool.tile([1, 2, NCOLS], f32, tag="r")
                nc.scalar.copy(out=r[:, 0], in_=psx)
                nc.vector.tensor_copy(out=r[:, 1], in_=psq)
                nc.sync.dma_start(
                    out=out[i].rearrange("a b -> (a b)"),
                    in_=r.rearrange("p a b -> p (a b)"),
                )

            off = 0
            for i, k in enumerate(ks):
                kp = k // 2
                xt = data_pool.tile([P, kp, 2, NCOLS], f8, tag="xt")
                nc.sync.dma_start(
                    out=xt.rearrange("p a b c -> p (a b c)"),
                    in_=xin[:, off : off + k * NCOLS],
                )
                off += k * NCOLS

                sqt = sq_pool.tile([P, kp, 2, NCOLS], f8, tag="sq")
                # contiguous pair-range split across engines (strided column
                # splits measured 2.2x slower on DVE/GpSimd)
                credit[0] += kp * SA
                credit[1] += kp * SV
                nA = max(0, min(kp, round(credit[0])))
                credit[0] -= nA
                nV = max(0, min(kp - nA, round(credit[1])))
                credit[1] -= nV
                if nA > 0:
                    nc.scalar.activation(
                        out=sqt[:, :nA], in_=xt[:, :nA],
                        func=mybir.ActivationFunctionType.Square,
                    )
                if nV > 0:
                    nc.vector.tensor_mul(sqt[:, nA:nA + nV], xt[:, nA:nA + nV],
                                         xt[:, nA:nA + nV])
                if nA + nV < kp:
                    nc.gpsimd.tensor_mul(sqt[:, nA + nV:], xt[:, nA + nV:],
                                         xt[:, nA + nV:])

                psx = ps_pool.tile([1, NCOLS], f32, tag="px")
                psq = ps_pool.tile([1, NCOLS], f32, tag="pq")
                emit_mm(psx, xt, kp)
                pending_sq.append((i, kp, sqt, psq))
                pending_cp.append((i, psx, psq))
                if i >= 1:
                    emit_sq_matmuls()
                if i >= 2:
                    emit_copies()
            while pending_sq:
                emit_sq_matmuls()
            while pending_cp:
                emit_copies()
    nc.finalize()
    return nc


def _pack(x, lengths):
    """Sort rows by chunk count, deal round-robin over cores, pack fp8 slots.

    Returns (bufs, ks, rowmap): bufs[c] = float8 [P, totalF]; ks = per-slot
    even chunk counts (identical across cores); rowmap[c] = int32
    [NSLOTS, GROUP] batch-row of each slot column group.
    """
    nch = -(-lengths // P)                      # chunks per row, 1..16
    order = np.argsort(-nch, kind="stable")     # descending

    # slot i's rows across all cores = sorted positions [128*i, 128*(i+1))
    ks = []
    for i in range(NSLOTS):
        k = int(nch[order[i * NCORES * GROUP]])
        ks.append(min(T // P, k + (k & 1)))     # round odd up to even
    totalF = sum(k * NCOLS for k in ks)

    mask = (np.arange(T)[None, :] < lengths[:, None])
    xm8 = np.where(mask[:, :, None], x, 0.0).astype(ml_dtypes.float8_e4m3)

    bufs, rowmap = [], []
    for c in range(NCORES):
        buf = np.zeros((P, totalF), dtype=ml_dtypes.float8_e4m3)
        rm = np.zeros((NSLOTS, GROUP), dtype=np.int32)
        off = 0
        for i, k in enumerate(ks):
            kp = k // 2
            rows = order[c + NCORES * (i * GROUP + np.arange(GROUP))]
            rm[i] = rows
            blk = xm8[rows, : kp * 2 * P, :]              # [16, k*128, 32]
            blk = blk.reshape(GROUP, kp, 2, P, D)
            blk = blk.transpose(3, 1, 2, 0, 4)            # [128, kp, 2, 16, 32]
            buf[:, off : off + k * NCOLS] = blk.reshape(P, k * NCOLS)
            off += k * NCOLS
        bufs.append(buf)
        rowmap.append(rm)
    return bufs, ks, rowmap


def _mlp(feats, W1, b1, g1, be1, W2, b2, g2, be2, W3, b3):
    M = W1.shape[0]
    acc = np.zeros((feats.shape[0], W3.shape[1]), dtype=np.float32)
    for m in range(M):
        h = feats @ W1[m].T + b1[m]
        mu = h.mean(0)
        var = h.var(0)
        h = (h - mu) / np.sqrt(var + EPS) * g1[m] + be1[m]
        np.maximum(h, 0.0, out=h)
        h = h @ W2[m].T + b2[m]
        mu = h.mean(0)
        var = h.var(0)
        h = (h - mu) / np.sqrt(var + EPS) * g2[m] + be2[m]
        np.maximum(h, 0.0, out=h)
        acc += h @ W3[m].T + b3[m]
    return acc / np.float32(M)


def kernel(x, lengths, W1, b1, g1, be1, W2, b2, g2, be2, W3, b3):
    global LAST_RESULTS
    x = np.ascontiguousarray(np.asarray(x, dtype=np.float32))
    lengths = np.asarray(lengths).astype(np.int64)

    bufs, ks, rowmap = _pack(x, lengths)
    nc = _build_bass(ks)
    in_maps = [{"xin": bufs[c]} for c in range(NCORES)]
    trace = bool(int(os.environ.get("KERNEL_TRACE", "0")))
    r = run_bass_kernel_spmd(nc, in_maps, core_ids=list(range(NCORES)), trace=trace)
    LAST_RESULTS = r

    sums = np.zeros((B, D), dtype=np.float64)
    sumsqs = np.zeros((B, D), dtype=np.float64)
    for c in range(NCORES):
        res = np.asarray(r.results[c]["res"], dtype=np.float64)  # [NSLOTS, 2, 512]
        res = res.reshape(NSLOTS, 2, GROUP, D)
        rows = rowmap[c].reshape(-1)
        sums[rows] = res[:, 0].reshape(-1, D)
        sumsqs[rows] = res[:, 1].reshape(-1, D)

    cnt = lengths.astype(np.float64)[:, None]
    mean = sums / cnt
    var = (sumsqs - cnt * mean * mean) / (cnt - 1.0)
    std = np.sqrt(np.maximum(var, 0.0))
    last = x[np.arange(B), lengths - 1]
    feats = np.concatenate(
        [mean.astype(np.float32), std.astype(np.float32), last], axis=1
    )

    W1, b1, g1, be1, W2, b2, g2, be2, W3, b3 = (
        np.asarray(a, dtype=np.float32)
        for a in (W1, b1, g1, be1, W2, b2, g2, be2, W3, b3)
    )
    return _mlp(feats, W1, b1, g1, be1, W2, b2, g2, be2, W3, b3)


# revision 18
# speedup vs baseline: 1.1573x; 1.1573x over previous
"""Trainium2 Bass kernel for nn_EnsembleClassifier (ragged_sequence).

Strategy
--------
The memory-bound work is masked mean/std pooling over x [2048, 2048, 32]
(~0.5 GB f32). The host masks past-length timesteps to zero and quantizes x
to fp8-e4m3 (verified end-to-end rel err ~5e-3, 4x under the 2e-2 gate),
quartering HBM traffic. Rows are sorted by chunk count ceil(L/128) and dealt
round-robin over the 8 cores; each core gets 16 slots of 16 rows x 32 dims =
512 PSUM columns, with k_slot (up to 16) 128-timestep chunks accumulated in
PSUM per slot.

On each core, per slot (tile xs [128, k, 2, 512]; s=0 holds x, s=1 squares):
  - one HWDGE DMA streams the fp8 block into the s=0 lanes,
  - squares are written into the s=1 lanes, split across ScalarE / VectorE /
    GpSimd by chunk range (shares tuned to measured rates: ~1.0 / 2.0 / 3.5
    ns per element),
  - TensorE runs ONE DoubleRow fp8 matmul chain per slot with selector
    weights [[1,0],[0,1]]: pair member 0 (x) feeds output row 0, member 1
    (x^2) feeds row 1, so a single [2, 512] PSUM bank accumulates both
    sum and sum-of-squares at 2 fp8 columns/cycle,
  - VectorE copies the [2, 512] PSUM result to SBUF; a per-slot DMA writes
    it out.

The host combines per-slot sums/sumsqs into masked mean/std, gathers the
last valid timestep from full-precision x, and runs the tiny 3-member MLP
ensemble with exact full-batch BatchNorm in numpy.
"""

import os

import ml_dtypes
import numpy as np

import concourse.bacc as bacc
import concourse.tile as tile
from concourse import mybir
from concourse.bass_utils import run_bass_kernel_spmd

B, T, D = 2048, 2048, 32
P = 128             # SBUF partitions = timesteps per chunk
NCORES = 8
GROUP = 16          # rows per slot (GROUP * D = 512 = PSUM bank f32 width)
NCOLS = GROUP * D   # 512
ROWS_PER_CORE = B // NCORES          # 256
NSLOTS = ROWS_PER_CORE // GROUP      # 16
EPS = 1e-5

# square-pass share per engine (contiguous chunk ranges, deficit-rounded):
# ScalarE / VectorE / GpSimd at measured ~1.0 / 2.0 / 3.5 ns per element
SA, SV = 0.60, 0.23

LAST_RESULTS = None


def _build_bass(ks):
    """ks: per-slot chunk counts (non-increasing), same for all cores."""
    nslots = len(ks)
    nc = bacc.Bacc()
    f32 = mybir.dt.float32
    f8 = mybir.dt.float8e4
    totalF = sum(k * NCOLS for k in ks)
    xin = nc.dram_tensor("xin", [P, totalF], f8, kind="ExternalInput")
    out = nc.dram_tensor("res", [nslots, 2, NCOLS], f32, kind="ExternalOutput")

    with tile.TileContext(nc) as tc:
        with (
            tc.tile_pool(name="wsel", bufs=1) as w_pool,
            tc.tile_pool(name="data", bufs=3) as data_pool,
            tc.tile_pool(name="ps", bufs=6, space="PSUM") as ps_pool,
            tc.tile_pool(name="resp", bufs=4) as res_pool,
        ):
            # DoubleRow selector weights: pair member 0 (x) -> out row 0,
            # member 1 (x^2) -> out row 1. Pair dim stride must be a
            # multiple of 16 bytes (s3_lw_dual_fp8_restrictions).
            w = w_pool.tile([P, 2, 16], f8)
            nc.vector.memset(w, 0.0)
            nc.vector.memset(w[:, 0:1, 0:1], 1.0)
            nc.vector.memset(w[:, 1:2, 1:2], 1.0)
            wsel = w[:, :, 0:2]

            pending_mm = []    # (slot, k, xs_tile)
            pending_cp = []    # (slot, ps_tile)
            credit = [0.0, 0.0]

            def emit_chain():
                i, k, xs = pending_mm.pop(0)
                ps = ps_pool.tile([2, NCOLS], f32, tag="ps")
                for j in range(k):
                    nc.tensor.matmul(
                        ps, wsel, xs[:, j], start=(j == 0), stop=(j == k - 1),
                        perf_mode=mybir.MatmulPerfMode.DoubleRow,
                    )
                pending_cp.append((i, ps))

            def emit_copy():
                i, ps = pending_cp.pop(0)
                r = res_pool.tile([2, NCOLS], f32, tag="r")
                nc.vector.tensor_copy(out=r, in_=ps)
                nc.sync.dma_start(
                    out=out[i].rearrange("a b -> (a b)"),
                    in_=r.rearrange("p b -> p (b)"),
                )

            off = 0
            for i, k in enumerate(ks):
                xs = data_pool.tile([P, k, 2, NCOLS], f8, tag="xs")
                nc.sync.dma_start(
                    out=xs[:, :, 0, :],
                    in_=xin[:, off : off + k * NCOLS],
                )
                off += k * NCOLS

                credit[0] += k * SA
                credit[1] += k * SV
                nA = max(0, min(k, round(credit[0])))
                credit[0] -= nA
                nV = max(0, min(k - nA, round(credit[1])))
                credit[1] -= nV
                if nA > 0:
                    nc.scalar.activation(
                        out=xs[:, :nA, 1, :], in_=xs[:, :nA, 0, :],
                        func=mybir.ActivationFunctionType.Square,
                    )
                if nV > 0:
                    nc.vector.tensor_mul(
                        xs[:, nA:nA + nV, 1, :],
                        xs[:, nA:nA + nV, 0, :], xs[:, nA:nA + nV, 0, :],
                    )
                if nA + nV < k:
                    nc.gpsimd.tensor_mul(
                        xs[:, nA + nV:, 1, :],
                        xs[:, nA + nV:, 0, :], xs[:, nA + nV:, 0, :],
                    )

                pending_mm.append((i, k, xs))
                if i >= 1:
                    emit_chain()
                if i >= 2:
                    emit_copy()
            while pending_mm:
                emit_chain()
            while pending_cp:
                emit_copy()
    nc.finalize()
    return nc


def _pack(x, lengths):
    """Sort rows by chunk count, deal round-robin over cores, pack fp8 slots.

    Returns (bufs, ks, rowmap): bufs[c] = float8 [P, totalF] (chunk-major
    [k, 512] per slot); ks = per-slot chunk counts (identical across cores);
    rowmap[c] = int32 [NSLOTS, GROUP] batch-row of each slot column group.
    """
    nch = -(-lengths // P)                      # chunks per row, 1..16
    order = np.argsort(-nch, kind="stable")     # descending

    # slot i's rows across all cores = sorted positions [128*i, 128*(i+1))
    ks = [int(nch[order[i * NCORES * GROUP]]) for i in range(NSLOTS)]
    totalF = sum(k * NCOLS for k in ks)

    mask = (np.arange(T)[None, :] < lengths[:, None])
    xm8 = np.where(mask[:, :, None], x, 0.0).astype(ml_dtypes.float8_e4m3)

    bufs, rowmap = [], []
    for c in range(NCORES):
        buf = np.zeros((P, totalF), dtype=ml_dtypes.float8_e4m3)
        rm = np.zeros((NSLOTS, GROUP), dtype=np.int32)
        off = 0
        for i, k in enumerate(ks):
            rows = order[c + NCORES * (i * GROUP + np.arange(GROUP))]
            rm[i] = rows
            blk = xm8[rows, : k * P, :]                   # [16, k*128, 32]
            blk = blk.reshape(GROUP, k, P, D)
            blk = blk.transpose(2, 1, 0, 3)               # [128, k, 16, 32]
            buf[:, off : off + k * NCOLS] = blk.reshape(P, k * NCOLS)
            off += k * NCOLS
        bufs.append(buf)
        rowmap.append(rm)
    return bufs, ks, rowmap


def _mlp(feats, W1, b1, g1, be1, W2, b2, g2, be2, W3, b3):
    M = W1.shape[0]
    acc = np.zeros((feats.shape[0], W3.shape[1]), dtype=np.float32)
    for m in range(M):
        h = feats @ W1[m].T + b1[m]
        mu = h.mean(0)
        var = h.var(0)
        h = (h - mu) / np.sqrt(var + EPS) * g1[m] + be1[m]
        np.maximum(h, 0.0, out=h)
        h = h @ W2[m].T + b2[m]
        mu = h.mean(0)
        var = h.var(0)
        h = (h - mu) / np.sqrt(var + EPS) * g2[m] + be2[m]
        np.maximum(h, 0.0, out=h)
        acc += h @ W3[m].T + b3[m]
    return acc / np.float32(M)


def kernel(x, lengths, W1, b1, g1, be1, W2, b2, g2, be2, W3, b3):
    global LAST_RESULTS
    x = np.ascontiguousarray(np.asarray(x, dtype=np.float32))
    lengths = np.asarray(lengths).astype(np.int64)

    bufs, ks, rowmap = _pack(x, lengths)
    nc = _build_bass(ks)
    in_maps = [{"xin": bufs[c]} for c in range(NCORES)]
    trace = bool(int(os.environ.get("KERNEL_TRACE", "0")))
    r = run_bass_kernel_spmd(nc, in_maps, core_ids=list(range(NCORES)), trace=trace)
    LAST_RESULTS = r

    sums = np.zeros((B, D), dtype=np.float64)
    sumsqs = np.zeros((B, D), dtype=np.float64)
    for c in range(NCORES):
        res = np.asarray(r.results[c]["res"], dtype=np.float64)  # [NSLOTS, 2, 512]
        res = res.reshape(NSLOTS, 2, GROUP, D)
        rows = rowmap[c].reshape(-1)
        sums[rows] = res[:, 0].reshape(-1, D)
        sumsqs[rows] = res[:, 1].reshape(-1, D)

    cnt = lengths.astype(np.float64)[:, None]
    mean = sums / cnt
    var = (sumsqs - cnt * mean * mean) / (cnt - 1.0)
    std = np.sqrt(np.maximum(var, 0.0))
    last = x[np.arange(B), lengths - 1]
    feats = np.concatenate(
        [mean.astype(np.float32), std.astype(np.float32), last], axis=1
    )

    W1, b1, g1, be1, W2, b2, g2, be2, W3, b3 = (
        np.asarray(a, dtype=np.float32)
        for a in (W1, b1, g1, be1, W2, b2, g2, be2, W3, b3)
    )
    return _mlp(feats, W1, b1, g1, be1, W2, b2, g2, be2, W3, b3)
